# revision 3
# baseline (speedup 1.0000x reference)
"""Transformer block (pre-LN causal MHA + MLP, plus head-averaged attention
probabilities) on 8 Trainium2 NeuronCores via Bass/Tile.

Shapes: x [2, 2048, 1024], H=16 heads, head_dim 64, MLP 1024->1024->1024.
Returns (x_out [2,2048,1024], a_mean [2,2048,2048]).

Strategy (fully token-parallel, zero replicated FLOPs):
  Launch 1 (one program, SPMD on 8 cores): each core takes 512 tokens
    (contiguous flat chunk). Feature-major LayerNorm (partition-dim stats
    via ones-matmuls on the PE), then Q^T/K^T (feature-major) and V
    (token-major) projections. LN affine (g1,b1) and biases are folded on
    the host; the K bias is dropped entirely (softmax is invariant to
    per-query score offsets, and (q+bq)·(k+bk) == (q+bq)·k + const(q)).
  Host: assembles per-batch K^T [1024,2048], head-major V [16,2048,64],
    Q^T, and re-slices queries into causally balanced block pairs
    {p, 7-p} of 256 tokens each.
  Launch 2 (4 program variants, one per block position p, each run on the
    2 cores holding that position for batch 0/1, all 4 pairs dispatched
    concurrently): causal attention entirely in the transposed
    orientation (scores^T = k_h^T q_h per 128-key tile; exp on ACT;
    denominators via ones-matmul partition reduction; in-place normalize;
    A@V accumulated into ctx^T), head-summed attention transposed back via
    the PE for the a_mean output, then o_proj + residual + LN2 + MLP
    (exact gelu) feature-major with weights as natural-layout lhsT, and a
    final PE transpose to emit token-major output rows.
  Matmuls run as float32r (1 cycle/row at free-dim>=256, ~1e-4 relative
  rounding); transposes and all vector/scalar math are plain fp32.
"""

import threading

import numpy as np
import jax
from jax.experimental.shard_map import shard_map
from jax.sharding import Mesh, PartitionSpec

import concourse.bass as bass  # noqa: F401  (registers engine classes)
import concourse.mybir as mybir
import concourse.tile as tile
from concourse import bacc
from concourse import bass2jax

R = mybir.dt.float32r
F = mybir.dt.float32
AF = mybir.ActivationFunctionType

B, S, D, H, HD = 2, 2048, 1024, 16, 64
TPC = 512            # tokens per core
PT = D // 128        # 8 partition tiles across the feature dim
NKT = S // 128       # 16 key tiles
EPS = 1e-5


# --------------------------------------------------------------------------
# shared layernorm (feature-major): out = (x - mu(x)) * rsqrt(var + eps)
# x, out: [128, PT, T] tiles; stats along the partition(feature) axis via
# ones-matmuls; per-token alpha/beta broadcast back with K=1 matmuls.
# --------------------------------------------------------------------------
def _layernorm_T(nc, sqp, smallp, ps_stat, ps_b, xt, out, onec, one128, eps_t, T):
    sq = sqp.tile([128, PT, T], R, tag="lnsq")
    for pt in range(PT):
        nc.vector.tensor_mul(sq[:, pt, :], xt[:, pt, :], xt[:, pt, :])
    sum_ps = ps_stat.tile([1, T], F, tag="lnsum")
    sum2_ps = ps_stat.tile([1, T], F, tag="lnsum")
    for pt in range(PT):
        nc.tensor.matmul(sum_ps, onec, xt[:, pt, :], start=(pt == 0), stop=(pt == PT - 1))
    for pt in range(PT):
        nc.tensor.matmul(sum2_ps, onec, sq[:, pt, :], start=(pt == 0), stop=(pt == PT - 1))
    mu = smallp.tile([1, T], F, tag="lnmu")
    nc.scalar.mul(mu, sum_ps, 1.0 / D)
    ex2 = smallp.tile([1, T], F, tag="lnex2")
    nc.scalar.mul(ex2, sum2_ps, 1.0 / D)
    var = smallp.tile([1, T], F, tag="lnvar")
    nc.vector.tensor_mul(var, mu, mu)
    nc.vector.tensor_sub(var, ex2, var)
    sd = smallp.tile([1, T], F, tag="lnsd")
    nc.scalar.activation(sd, var, AF.Sqrt, bias=eps_t, scale=1.0)
    alpha_f = smallp.tile([1, T], F, tag="lnalf")
    nc.vector.reciprocal(alpha_f, sd)
    alpha = smallp.tile([1, T], R, tag="lnal")
    nc.vector.tensor_copy(alpha, alpha_f)
    beta = smallp.tile([1, T], R, tag="lnbe")
    nc.vector.tensor_mul(beta, mu, alpha_f)
    nc.scalar.mul(beta, beta, -1.0)
    alpha_ps = ps_b.tile([128, T], F, tag="lnbc")
    nc.tensor.matmul(alpha_ps, one128, alpha, start=True, stop=True)
    beta_ps = ps_b.tile([128, T], F, tag="lnbc")
    nc.tensor.matmul(beta_ps, one128, beta, start=True, stop=True)
    for pt in range(PT):
        nc.vector.tensor_mul(out[:, pt, :], xt[:, pt, :], alpha_ps)
        nc.vector.tensor_add(out[:, pt, :], out[:, pt, :], beta_ps)


# --------------------------------------------------------------------------
# Launch 1: LN1 + QKV projections for a 512-token chunk.
# --------------------------------------------------------------------------
def _build_l1():
    nc = bacc.Bacc(None, target_bir_lowering=False)
    xT = nc.dram_tensor("xT", [D, TPC], R, kind="ExternalInput")
    wq = nc.dram_tensor("wq", [D, D], R, kind="ExternalInput")
    wk = nc.dram_tensor("wk", [D, D], R, kind="ExternalInput")
    wv = nc.dram_tensor("wv", [D, D], R, kind="ExternalInput")
    beq = nc.dram_tensor("beq", [1, D], R, kind="ExternalInput")
    bev = nc.dram_tensor("bev", [1, D], R, kind="ExternalInput")
    onec = nc.dram_tensor("onec", [128, 1], R, kind="ExternalInput")
    one128 = nc.dram_tensor("one128", [1, 128], R, kind="ExternalInput")
    one512 = nc.dram_tensor("one512", [1, TPC], R, kind="ExternalInput")
    qT_o = nc.dram_tensor("qT_o", [D, TPC], F, kind="ExternalOutput")
    kT_o = nc.dram_tensor("kT_o", [D, TPC], F, kind="ExternalOutput")
    v_o = nc.dram_tensor("v_o", [TPC, D], F, kind="ExternalOutput")

    with tile.TileContext(nc) as tc, nc.allow_low_precision("fp32r matmul pipeline"):
        with (
            tc.tile_pool(name="const", bufs=1) as constp,
            tc.tile_pool(name="big", bufs=1) as bigp,
            tc.tile_pool(name="small", bufs=1) as smallp,
            tc.tile_pool(name="wch", bufs=4) as wchp,
            tc.tile_pool(name="wvch", bufs=2) as wvchp,
            tc.tile_pool(name="stage", bufs=4) as stagep,
            tc.tile_pool(name="ps_stat", bufs=2, space="PSUM") as ps_stat,
            tc.tile_pool(name="ps_b", bufs=2, space="PSUM") as ps_b,
            tc.tile_pool(name="ps_mm", bufs=3, space="PSUM") as ps_mm,
        ):
            oc = constp.tile([128, 1], R)
            nc.sync.dma_start(out=oc, in_=onec[:, :])
            o128 = constp.tile([1, 128], R)
            nc.sync.dma_start(out=o128, in_=one128[:, :])
            o512 = constp.tile([1, TPC], R)
            nc.sync.dma_start(out=o512, in_=one512[:, :])
            beq_t = constp.tile([1, D], R)
            nc.sync.dma_start(out=beq_t, in_=beq[:, :])
            bev_t = constp.tile([1, D], R)
            nc.sync.dma_start(out=bev_t, in_=bev[:, :])
            eps_t = constp.tile([1, 1], F)
            nc.vector.memset(eps_t, EPS)

            xt = bigp.tile([128, PT, TPC], R, tag="xt")
            nc.sync.dma_start(out=xt, in_=xT.rearrange("(pt p) t -> p pt t", p=128))
            o = bigp.tile([128, PT, TPC], R, tag="o")
            _layernorm_T(nc, bigp, smallp, ps_stat, ps_b, xt, o, oc, o128, eps_t, TPC)

            # q^T / k^T: out[col, tok]; lhsT = W chunk (natural layout)
            for w_dram, bias_t, out_dram in ((wq, beq_t, qT_o), (wk, None, kT_o)):
                wre = w_dram.rearrange("(kt p) m -> p kt m", p=128)
                for mt in range(PT):
                    wc = wchp.tile([128, PT, 128], R, tag="wch")
                    nc.sync.dma_start(out=wc, in_=wre[:, :, mt * 128:(mt + 1) * 128])
                    ps = ps_mm.tile([128, TPC], F, tag="mm")
                    for kt in range(PT):
                        nc.tensor.matmul(
                            ps, wc[:, kt, :], o[:, kt, :],
                            start=(kt == 0),
                            stop=(kt == PT - 1 and bias_t is None),
                        )
                    if bias_t is not None:
                        nc.tensor.matmul(
                            ps, bias_t[0:1, mt * 128:(mt + 1) * 128], o512,
                            start=False, stop=True,
                        )
                    st = stagep.tile([128, TPC], F, tag="stage")
                    nc.scalar.copy(st, ps)
                    nc.sync.dma_start(out=out_dram[mt * 128:(mt + 1) * 128, :], in_=st)

            # v: out[tok, col]; lhsT = o^T chunk, rhs = W columns
            wvre = wv.rearrange("(kt p) m -> p kt m", p=128)
            for nh in range(2):
                wvc = wvchp.tile([128, PT, 512], R, tag="wv")
                nc.sync.dma_start(out=wvc, in_=wvre[:, :, nh * 512:(nh + 1) * 512])
                for tt in range(4):
                    ps = ps_mm.tile([128, 512], F, tag="mm")
                    for kt in range(PT):
                        nc.tensor.matmul(
                            ps, o[:, kt, tt * 128:(tt + 1) * 128], wvc[:, kt, :],
                            start=(kt == 0), stop=False,
                        )
                    nc.tensor.matmul(
                        ps, o128, bev_t[0:1, nh * 512:(nh + 1) * 512],
                        start=False, stop=True,
                    )
                    st = stagep.tile([128, 512], F, tag="stage")
                    nc.scalar.copy(st, ps)
                    nc.sync.dma_start(
                        out=v_o[tt * 128:(tt + 1) * 128, nh * 512:(nh + 1) * 512],
                        in_=st,
                    )
    nc.compile()
    return nc


# --------------------------------------------------------------------------
# Launch 2 (variant p in 0..3): causal attention for q-blocks {p, 7-p}
# (cols 0:256 and 256:512 of this core's 512 queries) + o_proj + LN2 + MLP.
# --------------------------------------------------------------------------
def _build_l2(p):
    ktA = 2 * p + 2       # key tiles covering q-block p   (cols 0:256)
    ktB = 16 - 2 * p      # key tiles covering q-block 7-p (cols 256:512)

    nc = bacc.Bacc(None, target_bir_lowering=False)
    qT = nc.dram_tensor("qT", [D, TPC], R, kind="ExternalInput")
    kT = nc.dram_tensor("kT", [D, S], R, kind="ExternalInput")
    vh = nc.dram_tensor("vh", [H, S, HD], R, kind="ExternalInput")
    xT = nc.dram_tensor("xT", [D, TPC], R, kind="ExternalInput")
    wo = nc.dram_tensor("wo", [D, D], R, kind="ExternalInput")
    wm1 = nc.dram_tensor("wm1", [D, D], R, kind="ExternalInput")
    wm2 = nc.dram_tensor("wm2", [D, D], R, kind="ExternalInput")
    bo = nc.dram_tensor("bo", [1, D], R, kind="ExternalInput")
    bem1 = nc.dram_tensor("bem1", [1, D], R, kind="ExternalInput")
    bm2 = nc.dram_tensor("bm2", [1, D], R, kind="ExternalInput")
    onec = nc.dram_tensor("onec", [128, 1], R, kind="ExternalInput")
    one128 = nc.dram_tensor("one128", [1, 128], R, kind="ExternalInput")
    one512 = nc.dram_tensor("one512", [1, TPC], R, kind="ExternalInput")
    ident = nc.dram_tensor("ident", [128, 128], F, kind="ExternalInput")
    out_o = nc.dram_tensor("out_o", [TPC, D], F, kind="ExternalOutput")
    am_o = nc.dram_tensor("am_o", [TPC, S], F, kind="ExternalOutput")

    with tile.TileContext(nc) as tc, nc.allow_low_precision("fp32r matmul pipeline"):
        with (
            tc.tile_pool(name="const", bufs=1) as constp,
            tc.tile_pool(name="small", bufs=1) as smallp,
            tc.tile_pool(name="den", bufs=2) as denp,
            tc.tile_pool(name="ctx", bufs=1) as ctxp,
        ):
            oc = constp.tile([128, 1], R)
            nc.sync.dma_start(out=oc, in_=onec[:, :])
            o128 = constp.tile([1, 128], R)
            nc.sync.dma_start(out=o128, in_=one128[:, :])
            o512 = constp.tile([1, TPC], R)
            nc.sync.dma_start(out=o512, in_=one512[:, :])
            ident_t = constp.tile([128, 128], F)
            nc.sync.dma_start(out=ident_t, in_=ident[:, :])
            bo_t = constp.tile([1, D], R)
            nc.sync.dma_start(out=bo_t, in_=bo[:, :])
            bem1_t = constp.tile([1, D], R)
            nc.sync.dma_start(out=bem1_t, in_=bem1[:, :])
            bm2_t = constp.tile([1, D], R)
            nc.sync.dma_start(out=bm2_t, in_=bm2[:, :])
            eps_t = constp.tile([1, 1], F)
            nc.vector.memset(eps_t, EPS)

            ctxT = ctxp.tile([128, PT, TPC], R, tag="ctxT")

            with tc.tile_pool(name="acc", bufs=1) as accp:
                acc = accp.tile([128, NKT, TPC], F, tag="acc")

                # ---------------- attention ----------------
                with (
                    tc.tile_pool(name="qt", bufs=1) as qtp,
                    tc.tile_pool(name="kp", bufs=2) as kvp,
                    tc.tile_pool(name="vp", bufs=2) as vvp,
                    tc.tile_pool(name="exp", bufs=18) as expp,
                    tc.tile_pool(name="rb", bufs=2) as rbp,
                    tc.tile_pool(name="ps_s", bufs=2, space="PSUM") as ps_s,
                    tc.tile_pool(name="ps_d", bufs=2, space="PSUM") as ps_d,
                    tc.tile_pool(name="ps_r", bufs=2, space="PSUM") as ps_r,
                    tc.tile_pool(name="ps_c", bufs=2, space="PSUM") as ps_c,
                ):
                    qt = qtp.tile([128, PT, TPC], R, tag="qt")
                    nc.sync.dma_start(out=qt, in_=qT.rearrange("(pt p) t -> p pt t", p=128))

                    for hp in range(H // 2):
                        kpair = kvp.tile([128, S], R, tag="k")
                        nc.sync.dma_start(out=kpair, in_=kT[hp * 128:(hp + 1) * 128, :])
                        vpair = vvp.tile([128, NKT, 128], R, tag="v")
                        nc.sync.dma_start(
                            out=vpair[:, :, 0:64],
                            in_=vh[2 * hp].rearrange("(kt p) d -> p kt d", p=128),
                        )
                        nc.sync.dma_start(
                            out=vpair[:, :, 64:128],
                            in_=vh[2 * hp + 1].rearrange("(kt p) d -> p kt d", p=128),
                        )
                        for hh in range(2):
                            lo, hi = hh * 64, hh * 64 + 64
                            es = []
                            for kt in range(ktB):
                                c0 = 0 if kt < ktA else 256
                                ps = ps_s.tile([128, 512], F, tag="s")
                                nc.tensor.matmul(
                                    ps[:, c0:512],
                                    kpair[lo:hi, kt * 128:(kt + 1) * 128],
                                    qt[lo:hi, hp, c0:512],
                                    start=True, stop=True,
                                )
                                e = expp.tile([128, 512], R, tag="e")
                                nc.scalar.activation(
                                    e[:, c0:512], ps[:, c0:512], AF.Exp, scale=0.125
                                )
                                if kt in (ktA - 2, ktA - 1):
                                    nc.gpsimd.affine_select(
                                        out=e[:, 0:256], in_=e[:, 0:256],
                                        compare_op=mybir.AluOpType.is_ge,
                                        fill=0.0, base=-(kt - (ktA - 2)) * 128,
                                        pattern=[[1, 256]], channel_multiplier=-1,
                                    )
                                if kt in (ktB - 2, ktB - 1):
                                    nc.gpsimd.affine_select(
                                        out=e[:, 256:512], in_=e[:, 256:512],
                                        compare_op=mybir.AluOpType.is_ge,
                                        fill=0.0, base=-(kt - (ktB - 2)) * 128,
                                        pattern=[[1, 256]], channel_multiplier=-1,
                                    )
                                es.append((kt, c0, e))

                            dA = ps_d.tile([1, 256], F, tag="d")
                            for kt in range(ktA):
                                nc.tensor.matmul(
                                    dA, oc, es[kt][2][:, 0:256],
                                    start=(kt == 0), stop=(kt == ktA - 1),
                                )
                            dB = ps_d.tile([1, 256], F, tag="d")
                            for kt in range(ktB):
                                nc.tensor.matmul(
                                    dB, oc, es[kt][2][:, 256:512],
                                    start=(kt == 0), stop=(kt == ktB - 1),
                                )
                            den = denp.tile([1, 512], F, tag="den")
                            nc.vector.tensor_copy(den[:, 0:256], dA)
                            nc.vector.tensor_copy(den[:, 256:512], dB)
                            rec = denp.tile([1, 512], R, tag="rec")
                            nc.vector.reciprocal(rec, den)
                            rb_ps = ps_r.tile([128, 512], F, tag="rb")
                            nc.tensor.matmul(rb_ps, o128, rec, start=True, stop=True)
                            rb = rbp.tile([128, 512], F, tag="rbs")
                            nc.scalar.copy(rb, rb_ps)

                            first = (hp == 0 and hh == 0)
                            for kt, c0, e in es:
                                nc.vector.tensor_mul(e[:, c0:512], e[:, c0:512], rb[:, c0:512])
                                if first:
                                    nc.vector.tensor_copy(acc[:, kt, c0:512], e[:, c0:512])
                                else:
                                    nc.vector.tensor_add(
                                        acc[:, kt, c0:512], acc[:, kt, c0:512], e[:, c0:512]
                                    )

                            ctx_ps = ps_c.tile([64, 512], F, tag="c")
                            for kt in range(ktA):
                                nc.tensor.matmul(
                                    ctx_ps[:, 0:256], vpair[:, kt, lo:hi], es[kt][2][:, 0:256],
                                    start=(kt == 0), stop=(kt == ktA - 1),
                                )
                            for kt in range(ktB):
                                nc.tensor.matmul(
                                    ctx_ps[:, 256:512], vpair[:, kt, lo:hi], es[kt][2][:, 256:512],
                                    start=(kt == 0), stop=(kt == ktB - 1),
                                )
                            nc.scalar.copy(ctxT[lo:hi, hp, :], ctx_ps)

                # ---------------- a_mean: transpose head-sum back ----------------
                with (
                    tc.tile_pool(name="amst", bufs=2) as amstp,
                    tc.tile_pool(name="ps_t", bufs=3, space="PSUM") as ps_t,
                ):
                    for qt_i in range(4):
                        lim = ktA if qt_i < 2 else ktB
                        stg = amstp.tile([128, NKT, 128], F, tag="am")
                        for kt in range(lim):
                            tp = ps_t.tile([128, 128], F, tag="t")
                            nc.tensor.transpose(
                                tp, acc[:, kt, qt_i * 128:(qt_i + 1) * 128], ident_t
                            )
                            nc.scalar.mul(stg[:, kt, :], tp, 1.0 / H)
                        nc.sync.dma_start(
                            out=am_o[qt_i * 128:(qt_i + 1) * 128, 0:lim * 128],
                            in_=stg[:, 0:lim, :],
                        )

            # ---------------- o_proj + residual + LN2 + MLP ----------------
            with tc.tile_pool(name="mbig", bufs=1) as mbigp:
              with (
                tc.tile_pool(name="wch", bufs=3) as wchp,
                tc.tile_pool(name="ps_m", bufs=3, space="PSUM") as ps_m,
                tc.tile_pool(name="ps_b2", bufs=2, space="PSUM") as ps_b2,
              ):
                xt = mbigp.tile([128, PT, TPC], R, tag="xt")
                nc.sync.dma_start(out=xt, in_=xT.rearrange("(pt p) t -> p pt t", p=128))

                x2 = mbigp.tile([128, PT, TPC], R, tag="x2")
                wore = wo.rearrange("(kt p) m -> p kt m", p=128)
                for mt in range(PT):
                    wc = wchp.tile([128, PT, 128], R, tag="wch")
                    nc.sync.dma_start(out=wc, in_=wore[:, :, mt * 128:(mt + 1) * 128])
                    ps = ps_m.tile([128, TPC], F, tag="m")
                    for kt in range(PT):
                        nc.tensor.matmul(
                            ps, wc[:, kt, :], ctxT[:, kt, :], start=(kt == 0), stop=False
                        )
                    nc.tensor.matmul(
                        ps, bo_t[0:1, mt * 128:(mt + 1) * 128], o512, start=False, stop=True
                    )
                    nc.vector.tensor_add(x2[:, mt, :], xt[:, mt, :], ps)

                x2n = mbigp.tile([128, PT, TPC], R, tag="x2n")
                _layernorm_T(nc, mbigp, smallp, ps_b2, ps_b2, x2, x2n, oc, o128, eps_t, TPC)

                h1g = mbigp.tile([128, PT, TPC], R, tag="h1g")
                wm1re = wm1.rearrange("(kt p) m -> p kt m", p=128)
                for mt in range(PT):
                    wc = wchp.tile([128, PT, 128], R, tag="wch")
                    nc.sync.dma_start(out=wc, in_=wm1re[:, :, mt * 128:(mt + 1) * 128])
                    ps = ps_m.tile([128, TPC], F, tag="m")
                    for kt in range(PT):
                        nc.tensor.matmul(
                            ps, wc[:, kt, :], x2n[:, kt, :], start=(kt == 0), stop=False
                        )
                    nc.tensor.matmul(
                        ps, bem1_t[0:1, mt * 128:(mt + 1) * 128], o512, start=False, stop=True
                    )
                    nc.scalar.activation(h1g[:, mt, :], ps, AF.Gelu)

                outT = mbigp.tile([128, PT, TPC], F, tag="outT")
                wm2re = wm2.rearrange("(kt p) m -> p kt m", p=128)
                for mt in range(PT):
                    wc = wchp.tile([128, PT, 128], R, tag="wch")
                    nc.sync.dma_start(out=wc, in_=wm2re[:, :, mt * 128:(mt + 1) * 128])
                    ps = ps_m.tile([128, TPC], F, tag="m")
                    for kt in range(PT):
                        nc.tensor.matmul(
                            ps, wc[:, kt, :], h1g[:, kt, :], start=(kt == 0), stop=False
                        )
                    nc.tensor.matmul(
                        ps, bm2_t[0:1, mt * 128:(mt + 1) * 128], o512, start=False, stop=True
                    )
                    nc.vector.tensor_add(outT[:, mt, :], x2[:, mt, :], ps)

              # transpose to token-major rows and store
              with (
                tc.tile_pool(name="orow", bufs=1) as orowp,
                tc.tile_pool(name="ps_t2", bufs=3, space="PSUM") as ps_t2,
              ):
                orow = orowp.tile([128, 4, D], F, tag="orow")
                for mt in range(PT):
                    for tt in range(4):
                        tp = ps_t2.tile([128, 128], F, tag="t2")
                        nc.tensor.transpose(
                            tp, outT[:, mt, tt * 128:(tt + 1) * 128], ident_t
                        )
                        nc.vector.tensor_copy(orow[:, tt, mt * 128:(mt + 1) * 128], tp)
                nc.sync.dma_start(
                    out=out_o.rearrange("(tt p) d -> p tt d", p=128), in_=orow
                )
    nc.compile()
    return nc


# --------------------------------------------------------------------------
# Cached PJRT runners (jit + NEFF compile happen once per process).
# --------------------------------------------------------------------------
_PROGS = {}
_JIT = {}
_BUILD_LOCK = threading.Lock()


def _ensure_built():
    with _BUILD_LOCK:
        if "l1" not in _PROGS:
            _PROGS["l1"] = _build_l1()
            for p in range(4):
                _PROGS[f"l2_{p}"] = _build_l2(p)
    return _PROGS


def _get_runner(name, nc, devices):
    key = (name, tuple(id(d) for d in devices))
    if key in _JIT:
        return _JIT[key]
    bass2jax.install_neuronx_cc_hook()
    n_cores = len(devices)
    partition_name = nc.partition_id_tensor.name if nc.partition_id_tensor else None
    in_names, out_names, out_avals = [], [], []
    for alloc in nc.m.functions[0].allocations:
        if not isinstance(alloc, mybir.MemoryLocationSet):
            continue
        nm = alloc.memorylocations[0].name
        if alloc.kind == "ExternalInput":
            if nm != partition_name:
                in_names.append(nm)
        elif alloc.kind == "ExternalOutput":
            out_names.append(nm)
            out_avals.append(
                jax.core.ShapedArray(
                    tuple(alloc.tensor_shape), mybir.dt.np(alloc.dtype)
                )
            )
    n_params, n_outs = len(in_names), len(out_names)
    all_names = list(in_names) + list(out_names)
    if partition_name is not None:
        all_names.append(partition_name)

    def _body(*args):
        operands = list(args)
        if partition_name is not None:
            operands.append(bass2jax.partition_id_tensor())
        outs = bass2jax._bass_exec_p.bind(
            *operands,
            out_avals=tuple(out_avals),
            in_names=tuple(all_names),
            out_names=tuple(out_names),
            lowering_input_output_aliases=(),
            sim_require_finite=True,
            sim_require_nnan=True,
            nc=nc,
        )
        return tuple(outs)

    donate = tuple(range(n_params, n_params + n_outs))
    mesh = Mesh(np.asarray(devices), ("core",))
    in_specs = (PartitionSpec("core"),) * (n_params + n_outs)
    out_specs = (PartitionSpec("core"),) * n_outs
    fn = jax.jit(
        shard_map(_body, mesh=mesh, in_specs=in_specs, out_specs=out_specs, check_rep=False),
        donate_argnums=donate,
        keep_unused=True,
    )
    runner = (fn, in_names, out_names, out_avals, n_cores)
    _JIT[key] = runner
    return runner


def _run_group(name, nc, in_maps, devices):
    fn, in_names, out_names, out_avals, n_cores = _get_runner(name, nc, devices)
    assert len(in_maps) == n_cores
    per_core = [
        [np.ascontiguousarray(np.asarray(m[n], np.float32)) for n in in_names]
        for m in in_maps
    ]
    concat_in = [
        np.concatenate([per_core[c][i] for c in range(n_cores)], axis=0)
        for i in range(len(in_names))
    ]
    concat_zeros = [
        np.zeros((n_cores * a.shape[0], *a.shape[1:]), a.dtype) for a in out_avals
    ]
    out_arrs = fn(*concat_in, *concat_zeros)
    return [
        {
            n: np.asarray(out_arrs[i]).reshape(n_cores, *out_avals[i].shape)[c]
            for i, n in enumerate(out_names)
        }
        for c in range(n_cores)
    ]


# --------------------------------------------------------------------------
# Host orchestration.
# --------------------------------------------------------------------------
def _fold_weights(inputs):
    f = lambda k: np.asarray(inputs[k], np.float32)
    g1, b1 = f("g1"), f("b1")
    g2, b2 = f("g2"), f("b2")
    W = {}
    W["wq"] = np.ascontiguousarray(g1[:, None] * f("Wq"))
    W["wk"] = np.ascontiguousarray(g1[:, None] * f("Wk"))
    W["wv"] = np.ascontiguousarray(g1[:, None] * f("Wv"))
    W["beq"] = (b1 @ f("Wq") + f("bq")).reshape(1, D)
    W["bev"] = (b1 @ f("Wv") + f("bv")).reshape(1, D)
    W["wo"] = np.ascontiguousarray(f("Wo"))
    W["bo"] = f("bo").reshape(1, D)
    W["wm1"] = np.ascontiguousarray(g2[:, None] * f("Wm1"))
    W["bem1"] = (b2 @ f("Wm1") + f("bm1")).reshape(1, D)
    W["wm2"] = np.ascontiguousarray(f("Wm2"))
    W["bm2"] = f("bm2").reshape(1, D)
    return W


_CONSTS = {
    "onec": np.ones((128, 1), np.float32),
    "one128": np.ones((1, 128), np.float32),
    "one512": np.ones((1, TPC), np.float32),
    "ident": np.eye(128, dtype=np.float32),
}


def kernel(**inputs):
    progs = _ensure_built()
    x = np.asarray(inputs["x"], np.float32)
    W = _fold_weights(inputs)
    XT = [np.ascontiguousarray(x[b].T) for b in range(B)]
    devs = jax.devices()

    # ---- launch 1: LN1 + QKV on 8 cores ----
    l1_maps = []
    for c in range(8):
        b, j = divmod(c, 4)
        l1_maps.append({
            "xT": XT[b][:, j * TPC:(j + 1) * TPC],
            "wq": W["wq"], "wk": W["wk"], "wv": W["wv"],
            "beq": W["beq"], "bev": W["bev"],
            "onec": _CONSTS["onec"], "one128": _CONSTS["one128"],
            "one512": _CONSTS["one512"],
        })
    rs = _run_group("l1", progs["l1"], l1_maps, devs[:8])

    QT = [np.concatenate([rs[b * 4 + j]["qT_o"] for j in range(4)], axis=1) for b in range(B)]
    KT = [np.concatenate([rs[b * 4 + j]["kT_o"] for j in range(4)], axis=1) for b in range(B)]
    V = [np.concatenate([rs[b * 4 + j]["v_o"] for j in range(4)], axis=0) for b in range(B)]
    VH = [np.ascontiguousarray(V[b].reshape(S, H, HD).transpose(1, 0, 2)) for b in range(B)]

    # ---- launch 2: 4 variants, concurrent on device pairs {p, p+4} ----
    results = [None] * 8
    errors = []

    def run_variant(p):
        try:
            def cols(M):
                return np.ascontiguousarray(np.concatenate(
                    [M[:, p * 256:(p + 1) * 256], M[:, (7 - p) * 256:(8 - p) * 256]],
                    axis=1,
                ))
            in_maps = []
            for b in range(B):
                in_maps.append({
                    "qT": cols(QT[b]), "xT": cols(XT[b]),
                    "kT": KT[b], "vh": VH[b],
                    "wo": W["wo"], "wm1": W["wm1"], "wm2": W["wm2"],
                    "bo": W["bo"], "bem1": W["bem1"], "bm2": W["bm2"],
                    **_CONSTS,
                })
            outs = _run_group(
                f"l2_{p}", progs[f"l2_{p}"], in_maps, [devs[p], devs[p + 4]]
            )
            for b in range(B):
                results[b * 4 + p] = outs[b]
        except Exception as e:  # noqa: BLE001
            errors.append(e)

    threads = [threading.Thread(target=run_variant, args=(p,)) for p in range(4)]
    for t in threads:
        t.start()
    for t in threads:
        t.join()
    if errors:
        raise errors[0]

    # ---- assemble full outputs ----
    x_out = np.empty((B, S, D), np.float32)
    a_mean = np.zeros((B, S, S), np.float32)
    for b in range(B):
        for p in range(4):
            r = results[b * 4 + p]
            x_out[b, p * 256:(p + 1) * 256] = r["out_o"][0:256]
            x_out[b, (7 - p) * 256:(8 - p) * 256] = r["out_o"][256:512]
            a_mean[b, p * 256:(p + 1) * 256] = r["am_o"][0:256]
            a_mean[b, (7 - p) * 256:(8 - p) * 256] = r["am_o"][256:512]
    return x_out, a_mean


# revision 4
# speedup vs baseline: 1.1785x; 1.1785x over previous
"""Transformer block (pre-LN causal MHA + MLP, plus head-averaged attention
probabilities) on 8 Trainium2 NeuronCores via Bass/Tile.

Shapes: x [2, 2048, 1024], H=16 heads, head_dim 64, MLP 1024->1024->1024.
Returns (x_out [2,2048,1024], a_mean [2,2048,2048]).

Strategy (fully token-parallel, zero replicated FLOPs):
  Launch 1 (one program, SPMD on 8 cores): each core takes 512 tokens
    (contiguous flat chunk). Feature-major LayerNorm (partition-dim stats
    via ones-matmuls on the PE), then Q^T/K^T (feature-major) and V
    (token-major) projections. LN affine (g1,b1) and biases are folded on
    the host; the K bias is dropped (softmax is invariant to per-query
    score offsets) and the V bias is folded into bo (softmax rows sum to
    one, so A@(V + 1 bev^T) @ Wo == A@V@Wo + bev@Wo).
  Host: assembles per-batch K^T [1024,2048], head-major ones-augmented V
    [16,2048,65] (the ones column makes each A@V matmul also emit the
    softmax denominator row for free), Q^T, and re-slices queries into
    causally balanced block pairs {p, 7-p} of 256 tokens each.
  Launch 2 (4 program variants, one per block position p, each run on the
    2 cores holding that position for batch 0/1, all 4 pairs dispatched
    concurrently): causal attention entirely in the transposed
    orientation (scores^T = k_h^T q_h per 128-key tile, exp on ACT,
    denominator from the augmented AV row, reciprocal on the
    partition-broadcast tile so all 128 DVE lanes work), head-summed
    attention transposed back via the PE for the a_mean output, then
    o_proj + residual + LN2 + MLP (exact gelu) feature-major with weights
    as natural-layout lhsT, and a final PE transpose to emit token-major
    output rows. All per-column biases are applied on ACT/DVE (per-
    partition bias APs in the feature-major orientation), keeping the PE
    stream pure matmul so the HAM clock gate stays at 2.4 GHz.
  Matmuls run as float32r (single-pass fp32, ~1e-4 relative rounding);
  transposes and all vector/scalar math are plain fp32.
"""

import threading

import numpy as np
import jax
from jax.experimental.shard_map import shard_map
from jax.sharding import Mesh, PartitionSpec

import concourse.bass as bass  # noqa: F401  (registers engine classes)
import concourse.mybir as mybir
import concourse.tile as tile
from concourse import bacc
from concourse import bass2jax

R = mybir.dt.float32r
F = mybir.dt.float32
AF = mybir.ActivationFunctionType

B, S, D, H, HD = 2, 2048, 1024, 16, 64
TPC = 512            # tokens per core
PT = D // 128        # 8 partition tiles across the feature dim
NKT = S // 128       # 16 key tiles
EPS = 1e-5


def _bcast_mid(ap, n):
    """View a [P, X] access pattern as [P, n, X] with a step-0 middle dim."""
    return bass.AP(
        tensor=ap.tensor,
        offset=ap.offset,
        ap=[ap.ap[0], [0, n], ap.ap[1]],
    )


# --------------------------------------------------------------------------
# shared layernorm (feature-major): out = (x - mu(x)) * rsqrt(var + eps)
# --------------------------------------------------------------------------
def _layernorm_T(nc, sqp, smallp, ps_stat, ps_b, xt, out, onec, one128, eps_t, T):
    sq = sqp.tile([128, PT, T], R, tag="lnsq")
    for pt in range(PT):
        nc.vector.tensor_mul(sq[:, pt, :], xt[:, pt, :], xt[:, pt, :])
    sum_ps = ps_stat.tile([1, T], F, tag="lnsum")
    sum2_ps = ps_stat.tile([1, T], F, tag="lnsum")
    for pt in range(PT):
        nc.tensor.matmul(sum_ps, onec, xt[:, pt, :], start=(pt == 0), stop=(pt == PT - 1))
    for pt in range(PT):
        nc.tensor.matmul(sum2_ps, onec, sq[:, pt, :], start=(pt == 0), stop=(pt == PT - 1))
    mu = smallp.tile([1, T], F, tag="lnmu")
    nc.scalar.mul(mu, sum_ps, 1.0 / D)
    ex2 = smallp.tile([1, T], F, tag="lnex2")
    nc.scalar.mul(ex2, sum2_ps, 1.0 / D)
    var = smallp.tile([1, T], F, tag="lnvar")
    nc.vector.tensor_mul(var, mu, mu)
    nc.vector.tensor_sub(var, ex2, var)
    sd = smallp.tile([1, T], F, tag="lnsd")
    nc.scalar.activation(sd, var, AF.Sqrt, bias=eps_t, scale=1.0)
    alpha_f = smallp.tile([1, T], F, tag="lnalf")
    nc.vector.reciprocal(alpha_f, sd)
    alpha = smallp.tile([1, T], R, tag="lnal")
    nc.vector.tensor_copy(alpha, alpha_f)
    beta = smallp.tile([1, T], R, tag="lnbe")
    nc.vector.tensor_mul(beta, mu, alpha_f)
    nc.scalar.mul(beta, beta, -1.0)
    alpha_ps = ps_b.tile([128, T], F, tag="lnbc")
    nc.tensor.matmul(alpha_ps, one128, alpha, start=True, stop=True)
    beta_ps = ps_b.tile([128, T], F, tag="lnbc")
    nc.tensor.matmul(beta_ps, one128, beta, start=True, stop=True)
    for pt in range(PT):
        nc.vector.tensor_mul(out[:, pt, :], xt[:, pt, :], alpha_ps)
        nc.vector.tensor_add(out[:, pt, :], out[:, pt, :], beta_ps)


# --------------------------------------------------------------------------
# Launch 1: LN1 + QKV projections for a 512-token chunk.
# --------------------------------------------------------------------------
def _build_l1():
    nc = bacc.Bacc(None, target_bir_lowering=False)
    xT = nc.dram_tensor("xT", [D, TPC], R, kind="ExternalInput")
    wq = nc.dram_tensor("wq", [D, D], R, kind="ExternalInput")
    wk = nc.dram_tensor("wk", [D, D], R, kind="ExternalInput")
    wv = nc.dram_tensor("wv", [D, D], R, kind="ExternalInput")
    beq_c = nc.dram_tensor("beq_c", [128, PT], F, kind="ExternalInput")
    onec = nc.dram_tensor("onec", [128, 1], R, kind="ExternalInput")
    one128 = nc.dram_tensor("one128", [1, 128], R, kind="ExternalInput")
    qT_o = nc.dram_tensor("qT_o", [D, TPC], F, kind="ExternalOutput")
    kT_o = nc.dram_tensor("kT_o", [D, TPC], F, kind="ExternalOutput")
    v_o = nc.dram_tensor("v_o", [TPC, D], F, kind="ExternalOutput")

    with tile.TileContext(nc) as tc, nc.allow_low_precision("fp32r matmul pipeline"):
        with (
            tc.tile_pool(name="const", bufs=1) as constp,
            tc.tile_pool(name="big", bufs=1) as bigp,
            tc.tile_pool(name="small", bufs=1) as smallp,
            tc.tile_pool(name="wch", bufs=4) as wchp,
            tc.tile_pool(name="wvch", bufs=2) as wvchp,
            tc.tile_pool(name="stage", bufs=4) as stagep,
            tc.tile_pool(name="ps_stat", bufs=2, space="PSUM") as ps_stat,
            tc.tile_pool(name="ps_b", bufs=2, space="PSUM") as ps_b,
            tc.tile_pool(name="ps_mm", bufs=3, space="PSUM") as ps_mm,
        ):
            oc = constp.tile([128, 1], R)
            nc.sync.dma_start(out=oc, in_=onec[:, :])
            o128 = constp.tile([1, 128], R)
            nc.sync.dma_start(out=o128, in_=one128[:, :])
            beq_t = constp.tile([128, PT], F)
            nc.sync.dma_start(out=beq_t, in_=beq_c[:, :])
            eps_t = constp.tile([1, 1], F)
            nc.vector.memset(eps_t, EPS)

            xt = bigp.tile([128, PT, TPC], R, tag="xt")
            nc.sync.dma_start(out=xt, in_=xT.rearrange("(pt p) t -> p pt t", p=128))
            o = bigp.tile([128, PT, TPC], R, tag="o")
            _layernorm_T(nc, bigp, smallp, ps_stat, ps_b, xt, o, oc, o128, eps_t, TPC)

            # q^T / k^T: out[col, tok]; lhsT = W chunk (natural layout)
            for w_dram, has_bias, out_dram in ((wq, True, qT_o), (wk, False, kT_o)):
                wre = w_dram.rearrange("(kt p) m -> p kt m", p=128)
                for mt in range(PT):
                    wc = wchp.tile([128, PT, 128], R, tag="wch")
                    nc.sync.dma_start(out=wc, in_=wre[:, :, mt * 128:(mt + 1) * 128])
                    ps = ps_mm.tile([128, TPC], F, tag="mm")
                    for kt in range(PT):
                        nc.tensor.matmul(
                            ps, wc[:, kt, :], o[:, kt, :],
                            start=(kt == 0), stop=(kt == PT - 1),
                        )
                    st = stagep.tile([128, TPC], F, tag="stage")
                    if has_bias:
                        nc.vector.tensor_scalar_add(st, ps, beq_t[:, mt:mt + 1])
                    else:
                        nc.scalar.copy(st, ps)
                    nc.sync.dma_start(out=out_dram[mt * 128:(mt + 1) * 128, :], in_=st)

            # v: out[tok, col]; lhsT = o^T chunk, rhs = W columns (bias folded
            # into bo on the host via softmax-row-sum-one)
            wvre = wv.rearrange("(kt p) m -> p kt m", p=128)
            for nh in range(2):
                wvc = wvchp.tile([128, PT, 512], R, tag="wv")
                nc.sync.dma_start(out=wvc, in_=wvre[:, :, nh * 512:(nh + 1) * 512])
                for tt in range(4):
                    ps = ps_mm.tile([128, 512], F, tag="mm")
                    for kt in range(PT):
                        nc.tensor.matmul(
                            ps, o[:, kt, tt * 128:(tt + 1) * 128], wvc[:, kt, :],
                            start=(kt == 0), stop=(kt == PT - 1),
                        )
                    st = stagep.tile([128, 512], F, tag="stage")
                    nc.scalar.copy(st, ps)
                    nc.sync.dma_start(
                        out=v_o[tt * 128:(tt + 1) * 128, nh * 512:(nh + 1) * 512],
                        in_=st,
                    )
    nc.compile()
    return nc


# --------------------------------------------------------------------------
# Launch 2 (variant p in 0..3): causal attention for q-blocks {p, 7-p}
# (cols 0:256 and 256:512 of this core's 512 queries) + o_proj + LN2 + MLP.
# --------------------------------------------------------------------------
def _build_l2(p):
    ktA = 2 * p + 2       # key tiles covering q-block p   (cols 0:256)
    ktB = 16 - 2 * p      # key tiles covering q-block 7-p (cols 256:512)

    nc = bacc.Bacc(None, target_bir_lowering=False)
    qT = nc.dram_tensor("qT", [D, TPC], R, kind="ExternalInput")
    kT = nc.dram_tensor("kT", [D, S], R, kind="ExternalInput")
    vh = nc.dram_tensor("vh", [H, S, HD + 1], R, kind="ExternalInput")
    xT = nc.dram_tensor("xT", [D, TPC], R, kind="ExternalInput")
    wo = nc.dram_tensor("wo", [D, D], R, kind="ExternalInput")
    wm1 = nc.dram_tensor("wm1", [D, D], R, kind="ExternalInput")
    wm2 = nc.dram_tensor("wm2", [D, D], R, kind="ExternalInput")
    bo_c = nc.dram_tensor("bo_c", [128, PT], F, kind="ExternalInput")
    bem1_c = nc.dram_tensor("bem1_c", [128, PT], F, kind="ExternalInput")
    bm2_c = nc.dram_tensor("bm2_c", [128, PT], F, kind="ExternalInput")
    onec = nc.dram_tensor("onec", [128, 1], R, kind="ExternalInput")
    one128 = nc.dram_tensor("one128", [1, 128], R, kind="ExternalInput")
    ident = nc.dram_tensor("ident", [128, 128], F, kind="ExternalInput")
    out_o = nc.dram_tensor("out_o", [TPC, D], F, kind="ExternalOutput")
    am_o = nc.dram_tensor("am_o", [TPC, S], F, kind="ExternalOutput")

    with tile.TileContext(nc) as tc, nc.allow_low_precision("fp32r matmul pipeline"):
        with (
            tc.tile_pool(name="const", bufs=1) as constp,
            tc.tile_pool(name="small", bufs=1) as smallp,
            tc.tile_pool(name="ctx", bufs=1) as ctxp,
        ):
            oc = constp.tile([128, 1], R)
            nc.sync.dma_start(out=oc, in_=onec[:, :])
            o128 = constp.tile([1, 128], R)
            nc.sync.dma_start(out=o128, in_=one128[:, :])
            ident_t = constp.tile([128, 128], F)
            nc.sync.dma_start(out=ident_t, in_=ident[:, :])
            bo_t = constp.tile([128, PT], F)
            nc.sync.dma_start(out=bo_t, in_=bo_c[:, :])
            bem1_t = constp.tile([128, PT], F)
            nc.sync.dma_start(out=bem1_t, in_=bem1_c[:, :])
            bm2_t = constp.tile([128, PT], F)
            nc.sync.dma_start(out=bm2_t, in_=bm2_c[:, :])
            eps_t = constp.tile([1, 1], F)
            nc.vector.memset(eps_t, EPS)

            ctxT = ctxp.tile([128, PT, TPC], R, tag="ctxT")

            with tc.tile_pool(name="acc", bufs=1) as accp:
                acc = accp.tile([128, NKT, TPC], F, tag="acc")

                # ---------------- attention ----------------
                with (
                    tc.tile_pool(name="qt", bufs=1) as qtp,
                    tc.tile_pool(name="kp", bufs=2) as kvp,
                    tc.tile_pool(name="vp", bufs=2) as vvp,
                    tc.tile_pool(name="exp", bufs=10) as expp,
                    tc.tile_pool(name="rb", bufs=3) as rbp,
                    tc.tile_pool(name="den", bufs=3) as denp,
                    tc.tile_pool(name="ps_s", bufs=3, space="PSUM") as ps_s,
                    tc.tile_pool(name="ps_c", bufs=2, space="PSUM") as ps_c,
                ):
                    qt = qtp.tile([128, PT, TPC], R, tag="qt")
                    nc.sync.dma_start(out=qt, in_=qT.rearrange("(pt p) t -> p pt t", p=128))

                    for hp in range(H // 2):
                        kpair = kvp.tile([128, S], R, tag="k")
                        nc.sync.dma_start(out=kpair, in_=kT[hp * 128:(hp + 1) * 128, :])
                        vtiles = []
                        for hh in range(2):
                            vt = vvp.tile([128, NKT, HD + 1], R, tag="v")
                            nc.sync.dma_start(
                                out=vt,
                                in_=vh[2 * hp + hh].rearrange("(kt p) d -> p kt d", p=128),
                            )
                            vtiles.append(vt)
                        for hh in range(2):
                            lo, hi = hh * 64, hh * 64 + 64
                            vt = vtiles[hh]
                            # scores + exp, kt pairs (ktA, ktB are even so a
                            # pair never straddles a causal region boundary)
                            epairs = []
                            for t in range(ktB // 2):
                                k0 = 2 * t
                                c0 = 0 if k0 + 1 < ktA else 256
                                ps2 = ps_s.tile([128, 2, 512], F, tag="s")
                                for j in (0, 1):
                                    kt = k0 + j
                                    nc.tensor.matmul(
                                        ps2[:, j, c0:512],
                                        kpair[lo:hi, kt * 128:(kt + 1) * 128],
                                        qt[lo:hi, hp, c0:512],
                                        start=True, stop=True,
                                    )
                                e2 = expp.tile([128, 2, 512], R, tag="e")
                                nc.scalar.activation(
                                    e2[:, :, c0:512], ps2[:, :, c0:512], AF.Exp, scale=0.125
                                )
                                for j in (0, 1):
                                    kt = k0 + j
                                    if kt in (ktA - 2, ktA - 1):
                                        nc.gpsimd.affine_select(
                                            out=e2[:, j, 0:256], in_=e2[:, j, 0:256],
                                            compare_op=mybir.AluOpType.is_ge,
                                            fill=0.0, base=-(kt - (ktA - 2)) * 128,
                                            pattern=[[1, 256]], channel_multiplier=-1,
                                        )
                                    if kt in (ktB - 2, ktB - 1):
                                        nc.gpsimd.affine_select(
                                            out=e2[:, j, 256:512], in_=e2[:, j, 256:512],
                                            compare_op=mybir.AluOpType.is_ge,
                                            fill=0.0, base=-(kt - (ktB - 2)) * 128,
                                            pattern=[[1, 256]], channel_multiplier=-1,
                                        )
                                epairs.append((t, c0, e2))

                            # AV with ones-augmented V: row 64 = denominator
                            ctx_ps = ps_c.tile([HD + 1, 512], F, tag="c")
                            for t, c0, e2 in epairs:
                                for j in (0, 1):
                                    kt = 2 * t + j
                                    nc.tensor.matmul(
                                        ctx_ps[:, c0:512],
                                        vt[:, kt, :],
                                        e2[:, j, c0:512],
                                        start=(kt == 0), stop=(kt == ktB - 1),
                                        skip_group_check=True,
                                    )

                            den = denp.tile([1, 512], F, tag="den")
                            nc.scalar.copy(den, ctx_ps[64:65, :])
                            rbraw = rbp.tile([128, 512], F, tag="rbraw")
                            nc.gpsimd.partition_broadcast(rbraw, den)
                            rb = rbp.tile([128, 512], F, tag="rb")
                            nc.vector.reciprocal(rb, rbraw)

                            # normalized context straight into ctx^T
                            nc.vector.tensor_mul(
                                ctxT[lo:hi, hp, :], ctx_ps[0:64, :], rb[0:64, :]
                            )

                            # normalize probabilities in place + head-sum
                            first = (hp == 0 and hh == 0)
                            for t, c0, e2 in epairs:
                                nc.vector.tensor_mul(
                                    e2[:, :, c0:512], e2[:, :, c0:512],
                                    _bcast_mid(rb[:, c0:512], 2),
                                )
                                if first:
                                    nc.vector.tensor_copy(
                                        acc[:, 2 * t:2 * t + 2, c0:512], e2[:, :, c0:512]
                                    )
                                else:
                                    nc.vector.tensor_add(
                                        acc[:, 2 * t:2 * t + 2, c0:512],
                                        acc[:, 2 * t:2 * t + 2, c0:512],
                                        e2[:, :, c0:512],
                                    )

                # ---------------- a_mean: transpose head-sum back ----------------
                with (
                    tc.tile_pool(name="amst", bufs=2) as amstp,
                    tc.tile_pool(name="ps_t", bufs=3, space="PSUM") as ps_t,
                ):
                    for qt_i in range(4):
                        lim = ktA if qt_i < 2 else ktB
                        stg = amstp.tile([128, NKT, 128], F, tag="am")
                        for kt in range(lim):
                            tp = ps_t.tile([128, 128], F, tag="t")
                            nc.tensor.transpose(
                                tp, acc[:, kt, qt_i * 128:(qt_i + 1) * 128], ident_t
                            )
                            nc.scalar.mul(stg[:, kt, :], tp, 1.0 / H)
                        nc.sync.dma_start(
                            out=am_o[qt_i * 128:(qt_i + 1) * 128, 0:lim * 128],
                            in_=stg[:, 0:lim, :],
                        )

            # ---------------- o_proj + residual + LN2 + MLP ----------------
            with tc.tile_pool(name="mbig", bufs=1) as mbigp:
              with (
                tc.tile_pool(name="wch", bufs=3) as wchp,
                tc.tile_pool(name="ps_m", bufs=3, space="PSUM") as ps_m,
                tc.tile_pool(name="ps_b2", bufs=2, space="PSUM") as ps_b2,
              ):
                xt = mbigp.tile([128, PT, TPC], R, tag="xt")
                nc.sync.dma_start(out=xt, in_=xT.rearrange("(pt p) t -> p pt t", p=128))

                x2 = mbigp.tile([128, PT, TPC], R, tag="x2")
                wore = wo.rearrange("(kt p) m -> p kt m", p=128)
                for mt in range(PT):
                    wc = wchp.tile([128, PT, 128], R, tag="wch")
                    nc.sync.dma_start(out=wc, in_=wore[:, :, mt * 128:(mt + 1) * 128])
                    ps = ps_m.tile([128, TPC], F, tag="m")
                    for kt in range(PT):
                        nc.tensor.matmul(
                            ps, wc[:, kt, :], ctxT[:, kt, :],
                            start=(kt == 0), stop=(kt == PT - 1),
                        )
                    # x2 = (ps + bo) + xt
                    nc.vector.scalar_tensor_tensor(
                        out=x2[:, mt, :], in0=ps, scalar=bo_t[:, mt:mt + 1],
                        in1=xt[:, mt, :],
                        op0=mybir.AluOpType.add, op1=mybir.AluOpType.add,
                    )

                x2n = mbigp.tile([128, PT, TPC], R, tag="x2n")
                _layernorm_T(nc, mbigp, smallp, ps_b2, ps_b2, x2, x2n, oc, o128, eps_t, TPC)

                h1g = mbigp.tile([128, PT, TPC], R, tag="h1g")
                wm1re = wm1.rearrange("(kt p) m -> p kt m", p=128)
                for mt in range(PT):
                    wc = wchp.tile([128, PT, 128], R, tag="wch")
                    nc.sync.dma_start(out=wc, in_=wm1re[:, :, mt * 128:(mt + 1) * 128])
                    ps = ps_m.tile([128, TPC], F, tag="m")
                    for kt in range(PT):
                        nc.tensor.matmul(
                            ps, wc[:, kt, :], x2n[:, kt, :],
                            start=(kt == 0), stop=(kt == PT - 1),
                        )
                    nc.scalar.activation(
                        h1g[:, mt, :], ps, AF.Gelu, bias=bem1_t[:, mt:mt + 1], scale=1.0
                    )

                outT = mbigp.tile([128, PT, TPC], F, tag="outT")
                wm2re = wm2.rearrange("(kt p) m -> p kt m", p=128)
                for mt in range(PT):
                    wc = wchp.tile([128, PT, 128], R, tag="wch")
                    nc.sync.dma_start(out=wc, in_=wm2re[:, :, mt * 128:(mt + 1) * 128])
                    ps = ps_m.tile([128, TPC], F, tag="m")
                    for kt in range(PT):
                        nc.tensor.matmul(
                            ps, wc[:, kt, :], h1g[:, kt, :],
                            start=(kt == 0), stop=(kt == PT - 1),
                        )
                    nc.vector.scalar_tensor_tensor(
                        out=outT[:, mt, :], in0=ps, scalar=bm2_t[:, mt:mt + 1],
                        in1=x2[:, mt, :],
                        op0=mybir.AluOpType.add, op1=mybir.AluOpType.add,
                    )

              # transpose to token-major rows and store
              with (
                tc.tile_pool(name="orow", bufs=1) as orowp,
                tc.tile_pool(name="ps_t2", bufs=3, space="PSUM") as ps_t2,
              ):
                orow = orowp.tile([128, 4, D], F, tag="orow")
                for mt in range(PT):
                    for tt in range(4):
                        tp = ps_t2.tile([128, 128], F, tag="t2")
                        nc.tensor.transpose(
                            tp, outT[:, mt, tt * 128:(tt + 1) * 128], ident_t
                        )
                        nc.vector.tensor_copy(orow[:, tt, mt * 128:(mt + 1) * 128], tp)
                nc.sync.dma_start(
                    out=out_o.rearrange("(tt p) d -> p tt d", p=128), in_=orow
                )
    nc.compile()
    return nc


# --------------------------------------------------------------------------
# Cached PJRT runners (jit + NEFF compile happen once per process).
# --------------------------------------------------------------------------
_PROGS = {}
_JIT = {}
_BUILD_LOCK = threading.Lock()


def _ensure_built():
    with _BUILD_LOCK:
        if "l1" not in _PROGS:
            _PROGS["l1"] = _build_l1()
            for p in range(4):
                _PROGS[f"l2_{p}"] = _build_l2(p)
    return _PROGS


def _get_runner(name, nc, devices):
    key = (name, tuple(id(d) for d in devices))
    if key in _JIT:
        return _JIT[key]
    bass2jax.install_neuronx_cc_hook()
    n_cores = len(devices)
    partition_name = nc.partition_id_tensor.name if nc.partition_id_tensor else None
    in_names, out_names, out_avals = [], [], []
    for alloc in nc.m.functions[0].allocations:
        if not isinstance(alloc, mybir.MemoryLocationSet):
            continue
        nm = alloc.memorylocations[0].name
        if alloc.kind == "ExternalInput":
            if nm != partition_name:
                in_names.append(nm)
        elif alloc.kind == "ExternalOutput":
            out_names.append(nm)
            out_avals.append(
                jax.core.ShapedArray(
                    tuple(alloc.tensor_shape), mybir.dt.np(alloc.dtype)
                )
            )
    n_params, n_outs = len(in_names), len(out_names)
    all_names = list(in_names) + list(out_names)
    if partition_name is not None:
        all_names.append(partition_name)

    def _body(*args):
        operands = list(args)
        if partition_name is not None:
            operands.append(bass2jax.partition_id_tensor())
        outs = bass2jax._bass_exec_p.bind(
            *operands,
            out_avals=tuple(out_avals),
            in_names=tuple(all_names),
            out_names=tuple(out_names),
            lowering_input_output_aliases=(),
            sim_require_finite=True,
            sim_require_nnan=True,
            nc=nc,
        )
        return tuple(outs)

    donate = tuple(range(n_params, n_params + n_outs))
    mesh = Mesh(np.asarray(devices), ("core",))
    in_specs = (PartitionSpec("core"),) * (n_params + n_outs)
    out_specs = (PartitionSpec("core"),) * n_outs
    fn = jax.jit(
        shard_map(_body, mesh=mesh, in_specs=in_specs, out_specs=out_specs, check_rep=False),
        donate_argnums=donate,
        keep_unused=True,
    )
    runner = (fn, in_names, out_names, out_avals, n_cores)
    _JIT[key] = runner
    return runner


def _run_group(name, nc, in_maps, devices):
    fn, in_names, out_names, out_avals, n_cores = _get_runner(name, nc, devices)
    assert len(in_maps) == n_cores
    per_core = [
        [np.ascontiguousarray(np.asarray(m[n], np.float32)) for n in in_names]
        for m in in_maps
    ]
    concat_in = [
        np.concatenate([per_core[c][i] for c in range(n_cores)], axis=0)
        for i in range(len(in_names))
    ]
    concat_zeros = [
        np.zeros((n_cores * a.shape[0], *a.shape[1:]), a.dtype) for a in out_avals
    ]
    out_arrs = fn(*concat_in, *concat_zeros)
    return [
        {
            n: np.asarray(out_arrs[i]).reshape(n_cores, *out_avals[i].shape)[c]
            for i, n in enumerate(out_names)
        }
        for c in range(n_cores)
    ]


# --------------------------------------------------------------------------
# Host orchestration.
# --------------------------------------------------------------------------
def _colmajor(b):
    return np.ascontiguousarray(b.reshape(PT, 128).T)


def _fold_weights(inputs):
    f = lambda k: np.asarray(inputs[k], np.float32)
    g1, b1 = f("g1"), f("b1")
    g2, b2 = f("g2"), f("b2")
    W = {}
    W["wq"] = np.ascontiguousarray(g1[:, None] * f("Wq"))
    W["wk"] = np.ascontiguousarray(g1[:, None] * f("Wk"))
    W["wv"] = np.ascontiguousarray(g1[:, None] * f("Wv"))
    W["beq_c"] = _colmajor(b1 @ f("Wq") + f("bq"))
    bev = b1 @ f("Wv") + f("bv")
    W["wo"] = np.ascontiguousarray(f("Wo"))
    W["bo_c"] = _colmajor(f("bo") + bev @ f("Wo"))
    W["wm1"] = np.ascontiguousarray(g2[:, None] * f("Wm1"))
    W["bem1_c"] = _colmajor(b2 @ f("Wm1") + f("bm1"))
    W["wm2"] = np.ascontiguousarray(f("Wm2"))
    W["bm2_c"] = _colmajor(f("bm2"))
    return W


_CONSTS = {
    "onec": np.ones((128, 1), np.float32),
    "one128": np.ones((1, 128), np.float32),
    "ident": np.eye(128, dtype=np.float32),
}


def _l1_inmaps(W, XT):
    maps = []
    for c in range(8):
        b, j = divmod(c, 4)
        maps.append({
            "xT": XT[b][:, j * TPC:(j + 1) * TPC],
            "wq": W["wq"], "wk": W["wk"], "wv": W["wv"],
            "beq_c": W["beq_c"],
            "onec": _CONSTS["onec"], "one128": _CONSTS["one128"],
        })
    return maps


def _assemble_l1(rs):
    QT = [np.concatenate([rs[b * 4 + j]["qT_o"] for j in range(4)], axis=1) for b in range(B)]
    KT = [np.concatenate([rs[b * 4 + j]["kT_o"] for j in range(4)], axis=1) for b in range(B)]
    V = [np.concatenate([rs[b * 4 + j]["v_o"] for j in range(4)], axis=0) for b in range(B)]
    VH = []
    for b in range(B):
        vv = np.empty((H, S, HD + 1), np.float32)
        vv[:, :, :HD] = V[b].reshape(S, H, HD).transpose(1, 0, 2)
        vv[:, :, HD] = 1.0
        VH.append(vv)
    return QT, KT, VH


def _l2_inmaps(W, XT, QT, KT, VH, p):
    def cols(M):
        return np.ascontiguousarray(np.concatenate(
            [M[:, p * 256:(p + 1) * 256], M[:, (7 - p) * 256:(8 - p) * 256]], axis=1))
    maps = []
    for b in range(B):
        maps.append({
            "qT": cols(QT[b]), "xT": cols(XT[b]), "kT": KT[b], "vh": VH[b],
            "wo": W["wo"], "wm1": W["wm1"], "wm2": W["wm2"],
            "bo_c": W["bo_c"], "bem1_c": W["bem1_c"], "bm2_c": W["bm2_c"],
            "onec": _CONSTS["onec"], "one128": _CONSTS["one128"],
            "ident": _CONSTS["ident"],
        })
    return maps


def kernel(**inputs):
    progs = _ensure_built()
    x = np.asarray(inputs["x"], np.float32)
    W = _fold_weights(inputs)
    XT = [np.ascontiguousarray(x[b].T) for b in range(B)]
    devs = jax.devices()

    rs = _run_group("l1", progs["l1"], _l1_inmaps(W, XT), devs[:8])
    QT, KT, VH = _assemble_l1(rs)

    results = [None] * 8
    errors = []

    def run_variant(p):
        try:
            outs = _run_group(
                f"l2_{p}", progs[f"l2_{p}"],
                _l2_inmaps(W, XT, QT, KT, VH, p),
                [devs[p], devs[p + 4]],
            )
            for b in range(B):
                results[b * 4 + p] = outs[b]
        except Exception as e:  # noqa: BLE001
            errors.append(e)

    threads = [threading.Thread(target=run_variant, args=(p,)) for p in range(4)]
    for t in threads:
        t.start()
    for t in threads:
        t.join()
    if errors:
        raise errors[0]

    x_out = np.empty((B, S, D), np.float32)
    a_mean = np.zeros((B, S, S), np.float32)
    for b in range(B):
        for p in range(4):
            r = results[b * 4 + p]
            x_out[b, p * 256:(p + 1) * 256] = r["out_o"][0:256]
            x_out[b, (7 - p) * 256:(8 - p) * 256] = r["out_o"][256:512]
            a_mean[b, p * 256:(p + 1) * 256] = r["am_o"][0:256]
            a_mean[b, (7 - p) * 256:(8 - p) * 256] = r["am_o"][256:512]
    return x_out, a_mean


# revision 7
# speedup vs baseline: 1.2914x; 1.0957x over previous
"""Transformer block (pre-LN causal MHA + MLP, plus head-averaged attention
probabilities) on 8 Trainium2 NeuronCores via Bass/Tile.

Shapes: x [2, 2048, 1024], H=16 heads, head_dim 64, MLP 1024->1024->1024.
Returns (x_out [2,2048,1024], a_mean [2,2048,2048]).

Strategy (fully token-parallel, zero replicated FLOPs):
  Launch 1 (one program, SPMD on 8 cores): each core takes 512 tokens
    (contiguous flat chunk). Feature-major LayerNorm (partition-dim stats
    via ones-matmuls on the PE), then Q^T/K^T (feature-major) and V
    (token-major) projections. LN affine (g1,b1) and biases are folded on
    the host; the K bias is dropped (softmax is invariant to per-query
    score offsets) and the V bias is folded into bo (softmax rows sum to
    one, so A@(V + 1 bev^T) @ Wo == A@V@Wo + bev@Wo).
  Host: assembles per-batch K^T [1024,2048], head-major ones-augmented V
    [16,2048,65] (the ones column makes each A@V matmul also emit the
    softmax denominator row for free), Q^T, and re-slices queries into
    causally balanced block pairs {p, 7-p} of 256 tokens each.
  Launch 2 (4 program variants, one per block position p, each run on the
    2 cores holding that position for batch 0/1, all 4 pairs dispatched
    concurrently): causal attention entirely in the transposed
    orientation (scores^T = k_h^T q_h per 128-key tile, exp on ACT,
    denominator from the augmented AV row, reciprocal on the
    partition-broadcast tile so all 128 DVE lanes work), head-summed
    attention transposed back via the PE for the a_mean output, then
    o_proj + residual + LN2 + MLP (exact gelu) feature-major with weights
    as natural-layout lhsT, and a final PE transpose to emit token-major
    output rows. All per-column biases are applied on ACT/DVE (per-
    partition bias APs in the feature-major orientation), keeping the PE
    stream pure matmul so the HAM clock gate stays at 2.4 GHz.
  Matmuls run as float32r (single-pass fp32, ~1e-4 relative rounding);
  transposes and all vector/scalar math are plain fp32.
"""

import threading

import numpy as np
import jax
from jax.experimental.shard_map import shard_map
from jax.sharding import Mesh, PartitionSpec

import concourse.bass as bass  # noqa: F401  (registers engine classes)
import concourse.mybir as mybir
import concourse.tile as tile
from concourse import bacc
from concourse import bass2jax

R = mybir.dt.float32r
F = mybir.dt.float32
AF = mybir.ActivationFunctionType

B, S, D, H, HD = 2, 2048, 1024, 16, 64
TPC = 512            # tokens per core
PT = D // 128        # 8 partition tiles across the feature dim
NKT = S // 128       # 16 key tiles
EPS = 1e-5


def _bcast_mid(ap, n):
    """View a [P, X] access pattern as [P, n, X] with a step-0 middle dim."""
    return bass.AP(
        tensor=ap.tensor,
        offset=ap.offset,
        ap=[ap.ap[0], [0, n], ap.ap[1]],
    )


# --------------------------------------------------------------------------
# shared layernorm (feature-major): out = (x - mu(x)) * rsqrt(var + eps)
# --------------------------------------------------------------------------
def _layernorm_T(nc, sqp, smallp, ps_stat, ps_b, xt, out, onec, one128, eps_t, T):
    sq = sqp.tile([128, PT, T], R, tag="lnsq")
    for pt in range(PT):
        nc.vector.tensor_mul(sq[:, pt, :], xt[:, pt, :], xt[:, pt, :])
    sum_ps = ps_stat.tile([1, T], F, tag="lnsum")
    sum2_ps = ps_stat.tile([1, T], F, tag="lnsum")
    for pt in range(PT):
        nc.tensor.matmul(sum_ps, onec, xt[:, pt, :], start=(pt == 0), stop=(pt == PT - 1))
    for pt in range(PT):
        nc.tensor.matmul(sum2_ps, onec, sq[:, pt, :], start=(pt == 0), stop=(pt == PT - 1))
    mu = smallp.tile([1, T], F, tag="lnmu")
    nc.scalar.mul(mu, sum_ps, 1.0 / D)
    ex2 = smallp.tile([1, T], F, tag="lnex2")
    nc.scalar.mul(ex2, sum2_ps, 1.0 / D)
    var = smallp.tile([1, T], F, tag="lnvar")
    nc.vector.tensor_mul(var, mu, mu)
    nc.vector.tensor_sub(var, ex2, var)
    sd = smallp.tile([1, T], F, tag="lnsd")
    nc.scalar.activation(sd, var, AF.Sqrt, bias=eps_t, scale=1.0)
    alpha_f = smallp.tile([1, T], F, tag="lnalf")
    nc.vector.reciprocal_approx_fast(out=alpha_f, in_=sd)
    alpha = smallp.tile([1, T], R, tag="lnal")
    nc.vector.tensor_copy(alpha, alpha_f)
    beta = smallp.tile([1, T], R, tag="lnbe")
    nc.vector.tensor_mul(beta, mu, alpha_f)
    nc.scalar.mul(beta, beta, -1.0)
    alpha_ps = ps_b.tile([128, T], F, tag="lnbc")
    nc.tensor.matmul(alpha_ps, one128, alpha, start=True, stop=True)
    beta_ps = ps_b.tile([128, T], F, tag="lnbc")
    nc.tensor.matmul(beta_ps, one128, beta, start=True, stop=True)
    for pt in range(PT):
        nc.vector.tensor_mul(out[:, pt, :], xt[:, pt, :], alpha_ps)
        nc.vector.tensor_add(out[:, pt, :], out[:, pt, :], beta_ps)


# --------------------------------------------------------------------------
# Launch 1: LN1 + QKV projections for a 512-token chunk.
# --------------------------------------------------------------------------
def _build_l1():
    nc = bacc.Bacc(None, target_bir_lowering=False)
    xT = nc.dram_tensor("xT", [D, TPC], R, kind="ExternalInput")
    wq = nc.dram_tensor("wq", [D, D], R, kind="ExternalInput")
    wk = nc.dram_tensor("wk", [D, D], R, kind="ExternalInput")
    wv = nc.dram_tensor("wv", [D, D], R, kind="ExternalInput")
    beq_c = nc.dram_tensor("beq_c", [128, PT], F, kind="ExternalInput")
    onec = nc.dram_tensor("onec", [128, 1], R, kind="ExternalInput")
    one128 = nc.dram_tensor("one128", [1, 128], R, kind="ExternalInput")
    qT_o = nc.dram_tensor("qT_o", [D, TPC], F, kind="ExternalOutput")
    kT_o = nc.dram_tensor("kT_o", [D, TPC], F, kind="ExternalOutput")
    v_o = nc.dram_tensor("v_o", [TPC, D], F, kind="ExternalOutput")

    with tile.TileContext(nc) as tc, nc.allow_low_precision("fp32r matmul pipeline"):
        with (
            tc.tile_pool(name="const", bufs=1) as constp,
            tc.tile_pool(name="big", bufs=1) as bigp,
            tc.tile_pool(name="small", bufs=1) as smallp,
            tc.tile_pool(name="wch", bufs=4) as wchp,
            tc.tile_pool(name="wvch", bufs=2) as wvchp,
            tc.tile_pool(name="stage", bufs=4) as stagep,
            tc.tile_pool(name="ps_stat", bufs=2, space="PSUM") as ps_stat,
            tc.tile_pool(name="ps_b", bufs=2, space="PSUM") as ps_b,
            tc.tile_pool(name="ps_mm", bufs=3, space="PSUM") as ps_mm,
        ):
            oc = constp.tile([128, 1], R)
            nc.sync.dma_start(out=oc, in_=onec[:, :])
            o128 = constp.tile([1, 128], R)
            nc.sync.dma_start(out=o128, in_=one128[:, :])
            beq_t = constp.tile([128, PT], F)
            nc.sync.dma_start(out=beq_t, in_=beq_c[:, :])
            eps_t = constp.tile([1, 1], F)
            nc.vector.memset(eps_t, EPS)

            xt = bigp.tile([128, PT, TPC], R, tag="xt")
            nc.sync.dma_start(out=xt, in_=xT.rearrange("(pt p) t -> p pt t", p=128))
            o = bigp.tile([128, PT, TPC], R, tag="o")
            _layernorm_T(nc, bigp, smallp, ps_stat, ps_b, xt, o, oc, o128, eps_t, TPC)

            # q^T / k^T: out[col, tok]; lhsT = W chunk (natural layout)
            for w_dram, has_bias, out_dram in ((wq, True, qT_o), (wk, False, kT_o)):
                wre = w_dram.rearrange("(kt p) m -> p kt m", p=128)
                for mt in range(PT):
                    wc = wchp.tile([128, PT, 128], R, tag="wch")
                    nc.sync.dma_start(out=wc, in_=wre[:, :, mt * 128:(mt + 1) * 128])
                    ps = ps_mm.tile([128, TPC], F, tag="mm")
                    for kt in range(PT):
                        nc.tensor.matmul(
                            ps, wc[:, kt, :], o[:, kt, :],
                            start=(kt == 0), stop=(kt == PT - 1),
                        )
                    st = stagep.tile([128, TPC], F, tag="stage")
                    if has_bias:
                        nc.vector.tensor_scalar_add(st, ps, beq_t[:, mt:mt + 1])
                    else:
                        nc.scalar.copy(st, ps)
                    nc.sync.dma_start(out=out_dram[mt * 128:(mt + 1) * 128, :], in_=st)

            # v: out[tok, col]; lhsT = o^T chunk, rhs = W columns (bias folded
            # into bo on the host via softmax-row-sum-one)
            wvre = wv.rearrange("(kt p) m -> p kt m", p=128)
            for nh in range(2):
                wvc = wvchp.tile([128, PT, 512], R, tag="wv")
                nc.sync.dma_start(out=wvc, in_=wvre[:, :, nh * 512:(nh + 1) * 512])
                for tt in range(4):
                    ps = ps_mm.tile([128, 512], F, tag="mm")
                    for kt in range(PT):
                        nc.tensor.matmul(
                            ps, o[:, kt, tt * 128:(tt + 1) * 128], wvc[:, kt, :],
                            start=(kt == 0), stop=(kt == PT - 1),
                        )
                    st = stagep.tile([128, 512], F, tag="stage")
                    nc.scalar.copy(st, ps)
                    nc.sync.dma_start(
                        out=v_o[tt * 128:(tt + 1) * 128, nh * 512:(nh + 1) * 512],
                        in_=st,
                    )
    nc.compile()
    return nc


# --------------------------------------------------------------------------
# Launch 2 (variant p in 0..3): causal attention for q-blocks {p, 7-p}
# (cols 0:256 and 256:512 of this core's 512 queries) + o_proj + LN2 + MLP.
# --------------------------------------------------------------------------
def _build_l2(p):
    ktA = 2 * p + 2       # key tiles covering q-block p   (cols 0:256)
    ktB = 16 - 2 * p      # key tiles covering q-block 7-p (cols 256:512)

    nc = bacc.Bacc(None, target_bir_lowering=False)
    qT = nc.dram_tensor("qT", [D, TPC], R, kind="ExternalInput")
    kT = nc.dram_tensor("kT", [D, S], R, kind="ExternalInput")
    vh = nc.dram_tensor("vh", [H, S, HD + 1], R, kind="ExternalInput")
    xT = nc.dram_tensor("xT", [D, TPC], R, kind="ExternalInput")
    wo = nc.dram_tensor("wo", [D, D], R, kind="ExternalInput")
    wm1 = nc.dram_tensor("wm1", [D, D], R, kind="ExternalInput")
    wm2 = nc.dram_tensor("wm2", [D, D], R, kind="ExternalInput")
    bo_c = nc.dram_tensor("bo_c", [128, PT], F, kind="ExternalInput")
    bem1_c = nc.dram_tensor("bem1_c", [128, PT], F, kind="ExternalInput")
    bm2_c = nc.dram_tensor("bm2_c", [128, PT], F, kind="ExternalInput")
    onec = nc.dram_tensor("onec", [128, 1], R, kind="ExternalInput")
    one128 = nc.dram_tensor("one128", [1, 128], R, kind="ExternalInput")
    ident = nc.dram_tensor("ident", [128, 128], F, kind="ExternalInput")
    out_o = nc.dram_tensor("out_o", [TPC, D], F, kind="ExternalOutput")
    am_o = nc.dram_tensor("am_o", [TPC, S], F, kind="ExternalOutput")

    with tile.TileContext(nc) as tc, nc.allow_low_precision("fp32r matmul pipeline"):
        with (
            tc.tile_pool(name="const", bufs=1) as constp,
            tc.tile_pool(name="small", bufs=1) as smallp,
            tc.tile_pool(name="ctx", bufs=1) as ctxp,
        ):
            oc = constp.tile([128, 1], R)
            nc.sync.dma_start(out=oc, in_=onec[:, :])
            o128 = constp.tile([1, 128], R)
            nc.sync.dma_start(out=o128, in_=one128[:, :])
            ident_t = constp.tile([128, 128], F)
            nc.sync.dma_start(out=ident_t, in_=ident[:, :])
            bo_t = constp.tile([128, PT], F)
            nc.sync.dma_start(out=bo_t, in_=bo_c[:, :])
            bem1_t = constp.tile([128, PT], F)
            nc.sync.dma_start(out=bem1_t, in_=bem1_c[:, :])
            bm2_t = constp.tile([128, PT], F)
            nc.sync.dma_start(out=bm2_t, in_=bm2_c[:, :])
            eps_t = constp.tile([1, 1], F)
            nc.vector.memset(eps_t, EPS)

            ctxT = ctxp.tile([128, PT, TPC], R, tag="ctxT")

            with tc.tile_pool(name="acc", bufs=1) as accp:
                acc = accp.tile([128, NKT, TPC], F, tag="acc")

                # ---------------- attention ----------------
                with (
                    tc.tile_pool(name="qt", bufs=1) as qtp,
                    tc.tile_pool(name="kp", bufs=2) as kvp,
                    tc.tile_pool(name="vp", bufs=3) as vvp,
                    tc.tile_pool(name="exp", bufs=18) as expp,
                    tc.tile_pool(name="rb", bufs=3) as rbp,
                    tc.tile_pool(name="den", bufs=3) as denp,
                    tc.tile_pool(name="ps_s", bufs=3, space="PSUM") as ps_s,
                    tc.tile_pool(name="ps_c", bufs=2, space="PSUM") as ps_c,
                ):
                    qt = qtp.tile([128, PT, TPC], R, tag="qt")
                    nc.sync.dma_start(out=qt, in_=qT.rearrange("(pt p) t -> p pt t", p=128))

                    # pre-warm the PE clock gate while input DMAs land
                    wps = ps_s.tile([128, 2, 512], F, tag="s")
                    for _ in range(48):
                        nc.tensor.matmul(
                            wps[:, 0, 0:128], o128, o128, start=True, stop=True
                        )

                    def attn_tail(hp, hh, vt, epairs):
                        lo, hi = hh * 64, hh * 64 + 64
                        # AV with ones-augmented V: row 64 = denominator
                        ctx_ps = ps_c.tile([HD + 1, 512], F, tag="c")
                        for t, c0, e2 in epairs:
                            for j in (0, 1):
                                kt = 2 * t + j
                                nc.tensor.matmul(
                                    ctx_ps[:, c0:512],
                                    vt[:, kt, :],
                                    e2[:, j, c0:512],
                                    start=(kt == 0), stop=(kt == ktB - 1),
                                    skip_group_check=True,
                                )
                        den = denp.tile([1, 512], F, tag="den")
                        nc.scalar.copy(den, ctx_ps[64:65, :])
                        rbraw = rbp.tile([128, 512], F, tag="rbraw")
                        nc.gpsimd.partition_broadcast(rbraw, den)
                        rb = rbp.tile([128, 512], F, tag="rb")
                        nc.vector.reciprocal_approx_fast(out=rb, in_=rbraw)
                        # normalized context straight into ctx^T
                        nc.vector.tensor_mul(
                            ctxT[lo:hi, hp, :], ctx_ps[0:64, :], rb[0:64, :]
                        )
                        # normalize probabilities in place + head-sum
                        first = (hp == 0 and hh == 0)
                        for t, c0, e2 in epairs:
                            nc.vector.tensor_mul(
                                e2[:, :, c0:512], e2[:, :, c0:512],
                                _bcast_mid(rb[:, c0:512], 2),
                            )
                            if first:
                                nc.vector.tensor_copy(
                                    acc[:, 2 * t:2 * t + 2, c0:512], e2[:, :, c0:512]
                                )
                            else:
                                nc.vector.tensor_add(
                                    acc[:, 2 * t:2 * t + 2, c0:512],
                                    acc[:, 2 * t:2 * t + 2, c0:512],
                                    e2[:, :, c0:512],
                                )

                    pend = None
                    for hp in range(H // 2):
                        kpair = kvp.tile([128, S], R, tag="k")
                        nc.sync.dma_start(out=kpair, in_=kT[hp * 128:(hp + 1) * 128, :])
                        vtiles = []
                        for hh in range(2):
                            vt = vvp.tile([128, NKT, HD + 1], R, tag="v")
                            nc.sync.dma_start(
                                out=vt,
                                in_=vh[2 * hp + hh].rearrange("(kt p) d -> p kt d", p=128),
                            )
                            vtiles.append(vt)
                        for hh in range(2):
                            lo, hi = hh * 64, hh * 64 + 64
                            # scores + exp, kt pairs (ktA, ktB are even so a
                            # pair never straddles a causal region boundary)
                            epairs = []
                            for t in range(ktB // 2):
                                k0 = 2 * t
                                c0 = 0 if k0 + 1 < ktA else 256
                                ps2 = ps_s.tile([128, 2, 512], F, tag="s")
                                for j in (0, 1):
                                    kt = k0 + j
                                    nc.tensor.matmul(
                                        ps2[:, j, c0:512],
                                        kpair[lo:hi, kt * 128:(kt + 1) * 128],
                                        qt[lo:hi, hp, c0:512],
                                        start=True, stop=True,
                                    )
                                e2 = expp.tile([128, 2, 512], R, tag="e")
                                nc.scalar.activation(
                                    e2[:, :, c0:512], ps2[:, :, c0:512], AF.Exp, scale=0.125
                                )
                                for j in (0, 1):
                                    kt = k0 + j
                                    if kt in (ktA - 2, ktA - 1):
                                        nc.gpsimd.affine_select(
                                            out=e2[:, j, 0:256], in_=e2[:, j, 0:256],
                                            compare_op=mybir.AluOpType.is_ge,
                                            fill=0.0, base=-(kt - (ktA - 2)) * 128,
                                            pattern=[[1, 256]], channel_multiplier=-1,
                                        )
                                    if kt in (ktB - 2, ktB - 1):
                                        nc.gpsimd.affine_select(
                                            out=e2[:, j, 256:512], in_=e2[:, j, 256:512],
                                            compare_op=mybir.AluOpType.is_ge,
                                            fill=0.0, base=-(kt - (ktB - 2)) * 128,
                                            pattern=[[1, 256]], channel_multiplier=-1,
                                        )
                                epairs.append((t, c0, e2))
                            if pend is not None:
                                attn_tail(*pend)
                            pend = (hp, hh, vtiles[hh], epairs)
                    attn_tail(*pend)

                # ---------------- a_mean: transpose head-sum back ----------------
                with (
                    tc.tile_pool(name="amst", bufs=2) as amstp,
                    tc.tile_pool(name="ps_t", bufs=3, space="PSUM") as ps_t,
                ):
                    wps = ps_t.tile([128, 128], F, tag="t")
                    for _ in range(16):
                        nc.tensor.matmul(wps, o128, o128, start=True, stop=True)
                    for qt_i in range(4):
                        lim = ktA if qt_i < 2 else ktB
                        stg = amstp.tile([128, NKT, 128], F, tag="am")
                        for kt in range(lim):
                            tp = ps_t.tile([128, 128], F, tag="t")
                            nc.tensor.transpose(
                                tp, acc[:, kt, qt_i * 128:(qt_i + 1) * 128], ident_t
                            )
                            nc.scalar.mul(stg[:, kt, :], tp, 1.0 / H)
                        nc.sync.dma_start(
                            out=am_o[qt_i * 128:(qt_i + 1) * 128, 0:lim * 128],
                            in_=stg[:, 0:lim, :],
                        )

            # ---------------- o_proj + residual + LN2 + MLP ----------------
            with tc.tile_pool(name="mbig", bufs=1) as mbigp:
              with (
                tc.tile_pool(name="wch", bufs=3) as wchp,
                tc.tile_pool(name="ps_m", bufs=3, space="PSUM") as ps_m,
                tc.tile_pool(name="ps_b2", bufs=2, space="PSUM") as ps_b2,
              ):
                xt = mbigp.tile([128, PT, TPC], R, tag="xt")
                nc.sync.dma_start(out=xt, in_=xT.rearrange("(pt p) t -> p pt t", p=128))

                x2 = mbigp.tile([128, PT, TPC], R, tag="x2")
                wore = wo.rearrange("(kt p) m -> p kt m", p=128)
                for mt in range(PT):
                    wc = wchp.tile([128, PT, 128], R, tag="wch")
                    nc.sync.dma_start(out=wc, in_=wore[:, :, mt * 128:(mt + 1) * 128])
                    ps = ps_m.tile([128, TPC], F, tag="m")
                    for kt in range(PT):
                        nc.tensor.matmul(
                            ps, wc[:, kt, :], ctxT[:, kt, :],
                            start=(kt == 0), stop=(kt == PT - 1),
                        )
                    # x2 = (ps + bo) + xt
                    nc.vector.scalar_tensor_tensor(
                        out=x2[:, mt, :], in0=ps, scalar=bo_t[:, mt:mt + 1],
                        in1=xt[:, mt, :],
                        op0=mybir.AluOpType.add, op1=mybir.AluOpType.add,
                    )

                x2n = mbigp.tile([128, PT, TPC], R, tag="x2n")
                _layernorm_T(nc, mbigp, smallp, ps_b2, ps_b2, x2, x2n, oc, o128, eps_t, TPC)
                wps2 = ps_m.tile([128, TPC], F, tag="m")
                for _ in range(12):
                    nc.tensor.matmul(wps2[:, 0:128], o128, o128, start=True, stop=True)

                h1g = mbigp.tile([128, PT, TPC], R, tag="h1g")
                wm1re = wm1.rearrange("(kt p) m -> p kt m", p=128)
                for mt in range(PT):
                    wc = wchp.tile([128, PT, 128], R, tag="wch")
                    nc.sync.dma_start(out=wc, in_=wm1re[:, :, mt * 128:(mt + 1) * 128])
                    ps = ps_m.tile([128, TPC], F, tag="m")
                    for kt in range(PT):
                        nc.tensor.matmul(
                            ps, wc[:, kt, :], x2n[:, kt, :],
                            start=(kt == 0), stop=(kt == PT - 1),
                        )
                    nc.scalar.activation(
                        h1g[:, mt, :], ps, AF.Gelu, bias=bem1_t[:, mt:mt + 1], scale=1.0
                    )

                outT = mbigp.tile([128, PT, TPC], F, tag="outT")
                wm2re = wm2.rearrange("(kt p) m -> p kt m", p=128)
                for mt in range(PT):
                    wc = wchp.tile([128, PT, 128], R, tag="wch")
                    nc.sync.dma_start(out=wc, in_=wm2re[:, :, mt * 128:(mt + 1) * 128])
                    ps = ps_m.tile([128, TPC], F, tag="m")
                    for kt in range(PT):
                        nc.tensor.matmul(
                            ps, wc[:, kt, :], h1g[:, kt, :],
                            start=(kt == 0), stop=(kt == PT - 1),
                        )
                    nc.vector.scalar_tensor_tensor(
                        out=outT[:, mt, :], in0=ps, scalar=bm2_t[:, mt:mt + 1],
                        in1=x2[:, mt, :],
                        op0=mybir.AluOpType.add, op1=mybir.AluOpType.add,
                    )

              # transpose to token-major rows and store
              with (
                tc.tile_pool(name="orow", bufs=1) as orowp,
                tc.tile_pool(name="ps_t2", bufs=3, space="PSUM") as ps_t2,
              ):
                orow = orowp.tile([128, 4, D], F, tag="orow")
                for mt in range(PT):
                    for tt in range(4):
                        tp = ps_t2.tile([128, 128], F, tag="t2")
                        nc.tensor.transpose(
                            tp, outT[:, mt, tt * 128:(tt + 1) * 128], ident_t
                        )
                        nc.vector.tensor_copy(orow[:, tt, mt * 128:(mt + 1) * 128], tp)
                nc.sync.dma_start(
                    out=out_o.rearrange("(tt p) d -> p tt d", p=128), in_=orow
                )
    nc.compile()
    return nc


# --------------------------------------------------------------------------
# Cached PJRT runners (jit + NEFF compile happen once per process).
# --------------------------------------------------------------------------
_PROGS = {}
_JIT = {}
_BUILD_LOCK = threading.Lock()


def _ensure_built():
    with _BUILD_LOCK:
        if "l1" not in _PROGS:
            _PROGS["l1"] = _build_l1()
            for p in range(4):
                _PROGS[f"l2_{p}"] = _build_l2(p)
    return _PROGS


def _get_runner(name, nc, devices):
    key = (name, tuple(id(d) for d in devices))
    if key in _JIT:
        return _JIT[key]
    bass2jax.install_neuronx_cc_hook()
    n_cores = len(devices)
    partition_name = nc.partition_id_tensor.name if nc.partition_id_tensor else None
    in_names, out_names, out_avals = [], [], []
    for alloc in nc.m.functions[0].allocations:
        if not isinstance(alloc, mybir.MemoryLocationSet):
            continue
        nm = alloc.memorylocations[0].name
        if alloc.kind == "ExternalInput":
            if nm != partition_name:
                in_names.append(nm)
        elif alloc.kind == "ExternalOutput":
            out_names.append(nm)
            out_avals.append(
                jax.core.ShapedArray(
                    tuple(alloc.tensor_shape), mybir.dt.np(alloc.dtype)
                )
            )
    n_params, n_outs = len(in_names), len(out_names)
    all_names = list(in_names) + list(out_names)
    if partition_name is not None:
        all_names.append(partition_name)

    def _body(*args):
        operands = list(args)
        if partition_name is not None:
            operands.append(bass2jax.partition_id_tensor())
        outs = bass2jax._bass_exec_p.bind(
            *operands,
            out_avals=tuple(out_avals),
            in_names=tuple(all_names),
            out_names=tuple(out_names),
            lowering_input_output_aliases=(),
            sim_require_finite=True,
            sim_require_nnan=True,
            nc=nc,
        )
        return tuple(outs)

    donate = tuple(range(n_params, n_params + n_outs))
    mesh = Mesh(np.asarray(devices), ("core",))
    in_specs = (PartitionSpec("core"),) * (n_params + n_outs)
    out_specs = (PartitionSpec("core"),) * n_outs
    fn = jax.jit(
        shard_map(_body, mesh=mesh, in_specs=in_specs, out_specs=out_specs, check_rep=False),
        donate_argnums=donate,
        keep_unused=True,
    )
    runner = (fn, in_names, out_names, out_avals, n_cores)
    _JIT[key] = runner
    return runner


def _run_group(name, nc, in_maps, devices):
    fn, in_names, out_names, out_avals, n_cores = _get_runner(name, nc, devices)
    assert len(in_maps) == n_cores
    per_core = [
        [np.ascontiguousarray(np.asarray(m[n], np.float32)) for n in in_names]
        for m in in_maps
    ]
    concat_in = [
        np.concatenate([per_core[c][i] for c in range(n_cores)], axis=0)
        for i in range(len(in_names))
    ]
    concat_zeros = [
        np.zeros((n_cores * a.shape[0], *a.shape[1:]), a.dtype) for a in out_avals
    ]
    out_arrs = fn(*concat_in, *concat_zeros)
    return [
        {
            n: np.asarray(out_arrs[i]).reshape(n_cores, *out_avals[i].shape)[c]
            for i, n in enumerate(out_names)
        }
        for c in range(n_cores)
    ]


# --------------------------------------------------------------------------
# Host orchestration.
# --------------------------------------------------------------------------
def _colmajor(b):
    return np.ascontiguousarray(b.reshape(PT, 128).T)


def _fold_weights(inputs):
    f = lambda k: np.asarray(inputs[k], np.float32)
    g1, b1 = f("g1"), f("b1")
    g2, b2 = f("g2"), f("b2")
    W = {}
    W["wq"] = np.ascontiguousarray(g1[:, None] * f("Wq"))
    W["wk"] = np.ascontiguousarray(g1[:, None] * f("Wk"))
    W["wv"] = np.ascontiguousarray(g1[:, None] * f("Wv"))
    W["beq_c"] = _colmajor(b1 @ f("Wq") + f("bq"))
    bev = b1 @ f("Wv") + f("bv")
    W["wo"] = np.ascontiguousarray(f("Wo"))
    W["bo_c"] = _colmajor(f("bo") + bev @ f("Wo"))
    W["wm1"] = np.ascontiguousarray(g2[:, None] * f("Wm1"))
    W["bem1_c"] = _colmajor(b2 @ f("Wm1") + f("bm1"))
    W["wm2"] = np.ascontiguousarray(f("Wm2"))
    W["bm2_c"] = _colmajor(f("bm2"))
    return W


_CONSTS = {
    "onec": np.ones((128, 1), np.float32),
    "one128": np.ones((1, 128), np.float32),
    "ident": np.eye(128, dtype=np.float32),
}


def _l1_inmaps(W, XT):
    maps = []
    for c in range(8):
        b, j = divmod(c, 4)
        maps.append({
            "xT": XT[b][:, j * TPC:(j + 1) * TPC],
            "wq": W["wq"], "wk": W["wk"], "wv": W["wv"],
            "beq_c": W["beq_c"],
            "onec": _CONSTS["onec"], "one128": _CONSTS["one128"],
        })
    return maps


def _assemble_l1(rs):
    QT = [np.concatenate([rs[b * 4 + j]["qT_o"] for j in range(4)], axis=1) for b in range(B)]
    KT = [np.concatenate([rs[b * 4 + j]["kT_o"] for j in range(4)], axis=1) for b in range(B)]
    V = [np.concatenate([rs[b * 4 + j]["v_o"] for j in range(4)], axis=0) for b in range(B)]
    VH = []
    for b in range(B):
        vv = np.empty((H, S, HD + 1), np.float32)
        vv[:, :, :HD] = V[b].reshape(S, H, HD).transpose(1, 0, 2)
        vv[:, :, HD] = 1.0
        VH.append(vv)
    return QT, KT, VH


def _l2_inmaps(W, XT, QT, KT, VH, p):
    def cols(M):
        return np.ascontiguousarray(np.concatenate(
            [M[:, p * 256:(p + 1) * 256], M[:, (7 - p) * 256:(8 - p) * 256]], axis=1))
    maps = []
    for b in range(B):
        maps.append({
            "qT": cols(QT[b]), "xT": cols(XT[b]), "kT": KT[b], "vh": VH[b],
            "wo": W["wo"], "wm1": W["wm1"], "wm2": W["wm2"],
            "bo_c": W["bo_c"], "bem1_c": W["bem1_c"], "bm2_c": W["bm2_c"],
            "onec": _CONSTS["onec"], "one128": _CONSTS["one128"],
            "ident": _CONSTS["ident"],
        })
    return maps


def kernel(**inputs):
    progs = _ensure_built()
    x = np.asarray(inputs["x"], np.float32)
    W = _fold_weights(inputs)
    XT = [np.ascontiguousarray(x[b].T) for b in range(B)]
    devs = jax.devices()

    rs = _run_group("l1", progs["l1"], _l1_inmaps(W, XT), devs[:8])
    QT, KT, VH = _assemble_l1(rs)

    results = [None] * 8
    errors = []

    def run_variant(p):
        try:
            outs = _run_group(
                f"l2_{p}", progs[f"l2_{p}"],
                _l2_inmaps(W, XT, QT, KT, VH, p),
                [devs[p], devs[p + 4]],
            )
            for b in range(B):
                results[b * 4 + p] = outs[b]
        except Exception as e:  # noqa: BLE001
            errors.append(e)

    threads = [threading.Thread(target=run_variant, args=(p,)) for p in range(4)]
    for t in threads:
        t.start()
    for t in threads:
        t.join()
    if errors:
        raise errors[0]

    x_out = np.empty((B, S, D), np.float32)
    a_mean = np.zeros((B, S, S), np.float32)
    for b in range(B):
        for p in range(4):
            r = results[b * 4 + p]
            x_out[b, p * 256:(p + 1) * 256] = r["out_o"][0:256]
            x_out[b, (7 - p) * 256:(8 - p) * 256] = r["out_o"][256:512]
            a_mean[b, p * 256:(p + 1) * 256] = r["am_o"][0:256]
            a_mean[b, (7 - p) * 256:(8 - p) * 256] = r["am_o"][256:512]
    return x_out, a_mean
